# revision 2
# baseline (speedup 1.0000x reference)
"""Trainium2 Bass kernel for DiagonalMultiplySum.

out[b, o, s] = sum_i input[b, i, s] * diagonal[o, i, s]

Shapes (hardcoded): input (64, 256, 4096) f32, diagonal (256, 256, 4096) f32,
output (64, 256, 4096) f32.

Strategy: shard the size axis across 8 NeuronCores (512 positions per core).
Each position s is an independent matmul out[:, :, s] = diag[:, :, s] @ in[:, :, s]^T
with contraction over i (256 -> 2 chunks of 128 on the PE partition dim).

v2 changes vs the fp32 baseline:
  * bf16 I/O end-to-end (rel-err ~4e-3 << 2e-2 gate).  Halves HBM traffic to
    ~100.6 MB/core (DMA roofline ~281 us @ 358 GB/s) and enables FWL
    (fast weight load) + full-rate bf16 streaming on the PE.
  * Host-side pre-packing: DRAM tensors are laid out exactly as the SBUF
    tiles want them, so every dma_start is a single fully-contiguous
    128-partition transfer.
  * Output is drained from PSUM as bf16 (DVE cast copy), halving store
    traffic; host converts back to fp32.
  * Loads (diag+input) ride the SP HWDGE ring, stores the ACT ring, so
    stores never head-of-line block loads.
  * Small windows (W positions per chunk) keep the PE pacing with the DMA
    stream (no >3.4us PE-idle gaps -> HAM stays warm).
"""

import os
import sys

for _p in ("/opt/trn_rl_repo",):
    if _p not in sys.path and os.path.isdir(_p):
        sys.path.insert(0, _p)

import numpy as np
from ml_dtypes import bfloat16

BATCH = 64
OUT_C = 256
IN_C = 256
SIZE = 4096
N_CORES = 8
S = SIZE // N_CORES  # 512 positions per core
P = 128

W = int(os.environ.get("DMS_W", "8"))  # positions per window
NW = S // W
X_IN = 2 * W * BATCH  # per-partition bf16 elems per window of input
X_DG = W * 512  # per-partition bf16 elems per window of diagonal
X_OUT = 2 * W * BATCH

_NC_CACHE = {}


def _build_nc():
    import concourse.bass as bass
    import concourse.mybir as mybir
    import concourse.tile as tile
    from contextlib import ExitStack

    fp32 = mybir.dt.float32
    bf16 = mybir.dt.bfloat16
    nc = bass.Bass(trn_type="TRN2")

    # Host pre-packed layouts (see prepare_in_maps):
    #   input    [p, nw, (ic, s, b)]   p = i % 128, ic = i // 128
    #   diagonal [p, nw, (s, ob, ic, o)]  p = i % 128, o0 = ob*128 + o
    #   output   [p, nw, (ob, s, b)]   p = o % 128
    inp = nc.dram_tensor("input", [P, NW, X_IN], bf16, kind="ExternalInput")
    dg = nc.dram_tensor("diagonal", [P, NW, X_DG], bf16, kind="ExternalInput")
    out = nc.dram_tensor("output", [P, NW, X_OUT], bf16, kind="ExternalOutput")

    n_in_buf = int(os.environ.get("DMS_INBUF", "4"))
    n_dg_buf = int(os.environ.get("DMS_DGBUF", "4"))
    n_out_buf = int(os.environ.get("DMS_OUTBUF", "3"))
    n_ps_buf = int(os.environ.get("DMS_PSBUF", "8"))

    with tile.TileContext(nc) as tc, ExitStack() as ctx:
        in_pool = ctx.enter_context(tc.tile_pool(name="inp", bufs=n_in_buf))
        dg_pool = ctx.enter_context(tc.tile_pool(name="dgp", bufs=n_dg_buf))
        out_pool = ctx.enter_context(tc.tile_pool(name="outp", bufs=n_out_buf))
        ps_pool = ctx.enter_context(tc.tile_pool(name="psp", bufs=n_ps_buf, space="PSUM"))

        for w in range(NW):
            in_t = in_pool.tile([P, X_IN], bf16, name="in_t")
            nc.sync.dma_start(out=in_t, in_=inp[:, w, :])
            dg_t = dg_pool.tile([P, X_DG], bf16, name="dg_t")
            nc.sync.dma_start(out=dg_t, in_=dg[:, w, :])

            in_t4 = in_t.rearrange("p (ic s b) -> p ic s b", ic=2, b=BATCH)
            dg_t5 = dg_t.rearrange("p (s ob ic o) -> p s ob ic o", ob=2, ic=2, o=P)

            out_t = out_pool.tile([P, X_OUT], bf16, name="out_t")
            out_t4 = out_t.rearrange("p (ob s b) -> p ob s b", ob=2, b=BATCH)

            for ob in range(2):
                for s8 in range(W // 8):
                    ps = ps_pool.tile([P, 512], fp32, name="ps")
                    ps3 = ps.rearrange("p (q b) -> p q b", q=8)
                    for s_ib in range(8):
                        s_loc = s8 * 8 + s_ib
                        for ic in range(2):
                            nc.tensor.matmul(
                                ps3[:, s_ib, :],
                                dg_t5[:, s_loc, ob, ic, :],
                                in_t4[:, ic, s_loc, :],
                                start=(ic == 0),
                                stop=(ic == 1),
                            )
                    # drain bank: psum (s, b) fp32 -> out_t (ob, s, b) bf16
                    nc.vector.tensor_copy(
                        out_t4[:, ob, s8 * 8 : s8 * 8 + 8, :], ps3
                    )

            nc.scalar.dma_start(out=out[:, w, :], in_=out_t)

    _split_multi_waits(nc)
    return nc


def _split_multi_waits(nc):
    """Walrus codegen supports only ONE sync-wait per instruction.

    Tile emits multiple waits on some instructions; hoist all but the last
    onto same-engine NoOp instructions inserted immediately before the
    offender.  Per-engine in-order issue makes this exactly equivalent.
    """
    import concourse.mybir as mybir

    for f in nc.m.functions:
        for blk in f.blocks:
            new_list = []
            changed = False
            for inst in blk.instructions:
                si = inst.sync_info
                waits = list(si.on_wait) if si and si.on_wait else []
                if len(waits) > 1:
                    for w in waits[:-1]:
                        nop = mybir.InstNoOp(
                            name=nc.get_next_instruction_name(),
                            engine=inst.engine,
                            ins=[],
                            outs=[],
                            sync_info=mybir.SyncInfo(on_wait=[w], on_update=[]),
                        )
                        nc.register_instruction(nop)
                        new_list.append(nop)
                    si.on_wait = [waits[-1]]
                    changed = True
                new_list.append(inst)
            if changed:
                blk.instructions = new_list


def _get_nc():
    key = "nc"
    if key not in _NC_CACHE:
        _NC_CACHE[key] = _build_nc()
    return _NC_CACHE[key]


def prepare_in_maps(inp, dg):
    """Pack full fp32 inputs into per-core bf16 DRAM layouts."""
    in_maps = []
    for c in range(N_CORES):
        sl = slice(c * S, (c + 1) * S)
        # input [b, i, s] -> [p, nw, ic, s, b],  i = ic*128 + p
        in_c = (
            inp[:, :, sl]
            .reshape(BATCH, 2, P, NW, W)
            .transpose(2, 3, 1, 4, 0)
            .astype(bfloat16)
            .reshape(P, NW, X_IN)
        )
        # diagonal [o0, i, s] -> [p, nw, s, ob, ic, o],  o0 = ob*128 + o
        dg_c = (
            dg[:, :, sl]
            .reshape(2, P, 2, P, NW, W)
            .transpose(3, 4, 5, 0, 2, 1)
            .astype(bfloat16)
            .reshape(P, NW, X_DG)
        )
        in_maps.append(
            {
                "input": np.ascontiguousarray(in_c),
                "diagonal": np.ascontiguousarray(dg_c),
            }
        )
    return in_maps


def assemble_output(results):
    """Unpack per-core bf16 [p, nw, (ob, s, b)] outputs to full fp32 [b, o, s]."""
    out = np.empty((BATCH, OUT_C, SIZE), dtype=np.float32)
    for c in range(N_CORES):
        sl = slice(c * S, (c + 1) * S)
        o_c = np.asarray(results[c]["output"]).reshape(P, NW, 2, W, BATCH)
        # [p, nw, ob, s, b] -> [b, (ob p), (nw s)]
        out[:, :, sl] = (
            o_c.transpose(4, 2, 0, 1, 3).reshape(BATCH, OUT_C, S).astype(np.float32)
        )
    return out


def kernel(**inputs):
    inp = np.asarray(inputs["input"], dtype=np.float32)
    dg = np.asarray(inputs["diagonal"], dtype=np.float32)
    assert inp.shape == (BATCH, IN_C, SIZE), inp.shape
    assert dg.shape == (OUT_C, IN_C, SIZE), dg.shape

    from concourse.bass_utils import run_bass_kernel_spmd

    nc = _get_nc()
    in_maps = prepare_in_maps(inp, dg)
    res = run_bass_kernel_spmd(nc, in_maps, list(range(N_CORES)))
    return assemble_output(res.results)


# revision 3
# speedup vs baseline: 1.2120x; 1.2120x over previous
"""Trainium2 Bass kernel for DiagonalMultiplySum.

out[b, o, s] = sum_i input[b, i, s] * diagonal[o, i, s]

Shapes (hardcoded): input (64, 256, 4096) f32, diagonal (256, 256, 4096) f32,
output (64, 256, 4096) f32.

Strategy: shard the size axis across 8 NeuronCores (512 positions per core).
Each position s is an independent matmul out[:, :, s] = diag[:, :, s] @ in[:, :, s]^T
with contraction over i (256 -> 2 chunks of 128 on the PE partition dim).

The kernel is HBM-DMA bound (~100.6 MB/core bf16 @ ~358 GB/s ~ 281 us), so
everything is organized around keeping the DMA stream dense:
  * bf16 I/O end-to-end (rel-err ~3e-3 << 2e-2 gate).
  * Host-side pre-packing: input+diagonal for each window are packed into a
    SINGLE contiguous 128-partition DRAM region -> one dma_start per window.
  * Output drained from PSUM as bf16 (DVE cast copy); host converts to fp32.
  * Loads ride the SP HWDGE ring, stores the ACT ring.
  * Window schedule: small windows at the start (quick pipeline ramp) and
    end (short tail), large windows in the middle (per-transfer efficiency,
    fewer read/write turnarounds).
"""

import os
import sys

for _p in ("/opt/trn_rl_repo",):
    if _p not in sys.path and os.path.isdir(_p):
        sys.path.insert(0, _p)

import numpy as np
from ml_dtypes import bfloat16

BATCH = 64
OUT_C = 256
IN_C = 256
SIZE = 4096
N_CORES = 8
S = SIZE // N_CORES  # 512 positions per core
P = 128


def _parse_sched(spec):
    out = []
    for seg in spec.split(","):
        seg = seg.strip()
        if "*" in seg:
            w, n = seg.split("*")
            out.extend([int(w)] * int(n))
        else:
            out.append(int(seg))
    assert sum(out) == S, (out, sum(out))
    assert all(w % 8 == 0 for w in out)
    return out

# window schedule: positions per window
WS = _parse_sched(os.environ.get("DMS_WS", "8,8,32*15,8,8"))
NW = len(WS)
# per-window free-dim extents (bf16 elems per partition)
X_IN = [2 * w * BATCH for w in WS]
X_DG = [w * 512 for w in WS]
X_LD = [a + b for a, b in zip(X_IN, X_DG)]
X_OUT = [2 * w * BATCH for w in WS]
LD_OFF = np.concatenate([[0], np.cumsum(X_LD)]).astype(int)
OUT_OFF = np.concatenate([[0], np.cumsum(X_OUT)]).astype(int)
TOT_LD = int(LD_OFF[-1])
TOT_OUT = int(OUT_OFF[-1])
S_OFF = np.concatenate([[0], np.cumsum(WS)]).astype(int)

_NC_CACHE = {}


def _build_nc():
    import concourse.bass as bass
    import concourse.mybir as mybir
    import concourse.tile as tile
    from contextlib import ExitStack

    fp32 = mybir.dt.float32
    bf16 = mybir.dt.bfloat16
    nc = bass.Bass(trn_type="TRN2")

    # Host pre-packed layouts (see prepare_in_maps). Per window w the load
    # region holds  [ (ic, s, b) input | (s, ob, ic, o) diagonal ]  on each
    # of the 128 partitions (p = i % 128 for both halves).
    ld = nc.dram_tensor("loads", [P, TOT_LD], bf16, kind="ExternalInput")
    out = nc.dram_tensor("output", [P, TOT_OUT], bf16, kind="ExternalOutput")

    n_ld_buf = int(os.environ.get("DMS_LDBUF", "4"))
    n_out_buf = int(os.environ.get("DMS_OUTBUF", "3"))
    n_ps_buf = int(os.environ.get("DMS_PSBUF", "8"))
    wmax = max(WS)

    with tile.TileContext(nc) as tc, ExitStack() as ctx:
        ld_pool = ctx.enter_context(tc.tile_pool(name="ldp", bufs=n_ld_buf))
        out_pool = ctx.enter_context(tc.tile_pool(name="outp", bufs=n_out_buf))
        ps_pool = ctx.enter_context(tc.tile_pool(name="psp", bufs=n_ps_buf, space="PSUM"))

        for w in range(NW):
            W = WS[w]
            ld_t = ld_pool.tile([P, max(X_LD)], bf16, name="ld_t", tag="ld_t")
            nc.sync.dma_start(
                out=ld_t[:, 0 : X_LD[w]], in_=ld[:, LD_OFF[w] : LD_OFF[w + 1]]
            )

            in_t4 = ld_t[:, 0 : X_IN[w]].rearrange(
                "p (ic s b) -> p ic s b", ic=2, b=BATCH
            )
            dg_t5 = ld_t[:, X_IN[w] : X_LD[w]].rearrange(
                "p (s ob ic o) -> p s ob ic o", ob=2, ic=2, o=P
            )

            out_t = out_pool.tile([P, max(X_OUT)], bf16, name="out_t", tag="out_t")
            out_t4 = out_t[:, 0 : X_OUT[w]].rearrange(
                "p (ob s b) -> p ob s b", ob=2, b=BATCH
            )

            for ob in range(2):
                for s8 in range(W // 8):
                    ps = ps_pool.tile([P, 512], fp32, name="ps")
                    ps3 = ps.rearrange("p (q b) -> p q b", q=8)
                    for s_ib in range(8):
                        s_loc = s8 * 8 + s_ib
                        for ic in range(2):
                            nc.tensor.matmul(
                                ps3[:, s_ib, :],
                                dg_t5[:, s_loc, ob, ic, :],
                                in_t4[:, ic, s_loc, :],
                                start=(ic == 0),
                                stop=(ic == 1),
                            )
                    # drain bank: psum (s, b) fp32 -> out_t (ob, s, b) bf16
                    nc.vector.tensor_copy(
                        out_t4[:, ob, s8 * 8 : s8 * 8 + 8, :], ps3
                    )

            nc.scalar.dma_start(
                out=out[:, OUT_OFF[w] : OUT_OFF[w + 1]], in_=out_t[:, 0 : X_OUT[w]]
            )

    _split_multi_waits(nc)
    return nc


def _split_multi_waits(nc):
    """Walrus codegen supports only ONE sync-wait per instruction.

    Tile emits multiple waits on some instructions; hoist all but the last
    onto same-engine NoOp instructions inserted immediately before the
    offender.  Per-engine in-order issue makes this exactly equivalent.
    """
    import concourse.mybir as mybir

    for f in nc.m.functions:
        for blk in f.blocks:
            new_list = []
            changed = False
            for inst in blk.instructions:
                si = inst.sync_info
                waits = list(si.on_wait) if si and si.on_wait else []
                if len(waits) > 1:
                    for w in waits[:-1]:
                        nop = mybir.InstNoOp(
                            name=nc.get_next_instruction_name(),
                            engine=inst.engine,
                            ins=[],
                            outs=[],
                            sync_info=mybir.SyncInfo(on_wait=[w], on_update=[]),
                        )
                        nc.register_instruction(nop)
                        new_list.append(nop)
                    si.on_wait = [waits[-1]]
                    changed = True
                new_list.append(inst)
            if changed:
                blk.instructions = new_list


def _get_nc():
    key = "nc"
    if key not in _NC_CACHE:
        _NC_CACHE[key] = _build_nc()
    return _NC_CACHE[key]


def prepare_in_maps(inp, dg):
    """Pack full fp32 inputs into per-core bf16 DRAM load regions."""
    in_maps = []
    for c in range(N_CORES):
        sl = slice(c * S, (c + 1) * S)
        # input [b, i, s] -> [p, s, ...],  i = ic*128 + p
        in_c = (
            inp[:, :, sl]
            .reshape(BATCH, 2, P, S)
            .transpose(2, 1, 3, 0)
            .astype(bfloat16)
        )  # [p, ic, s, b]
        # diagonal [o0, i, s] -> [p, s, ob, ic, o],  o0 = ob*128 + o
        dg_c = (
            dg[:, :, sl]
            .reshape(2, P, 2, P, S)
            .transpose(3, 4, 0, 2, 1)
            .astype(bfloat16)
        )  # [p, s, ob, ic, o]
        comb = np.empty((P, TOT_LD), dtype=bfloat16)
        for w in range(NW):
            s0, s1 = S_OFF[w], S_OFF[w + 1]
            o0, o1 = LD_OFF[w], LD_OFF[w + 1]
            comb[:, o0 : o0 + X_IN[w]] = in_c[:, :, s0:s1, :].reshape(P, -1)
            comb[:, o0 + X_IN[w] : o1] = dg_c[:, s0:s1].reshape(P, -1)
        in_maps.append({"loads": comb})
    return in_maps


def assemble_output(results):
    """Unpack per-core bf16 [p, (w: ob, s, b)] outputs to full fp32 [b, o, s]."""
    out = np.empty((BATCH, OUT_C, SIZE), dtype=np.float32)
    for c in range(N_CORES):
        sl = slice(c * S, (c + 1) * S)
        o_c = np.asarray(results[c]["output"])  # [P, TOT_OUT]
        o_full = np.empty((P, 2, S, BATCH), dtype=np.float32)  # [p, ob, s, b]
        for w in range(NW):
            s0, s1 = S_OFF[w], S_OFF[w + 1]
            blk = o_c[:, OUT_OFF[w] : OUT_OFF[w + 1]].reshape(P, 2, WS[w], BATCH)
            o_full[:, :, s0:s1, :] = blk.astype(np.float32)
        # [p, ob, s, b] -> [b, (ob p), s]
        out[:, :, sl] = o_full.transpose(3, 1, 0, 2).reshape(BATCH, OUT_C, S)
    return out


def kernel(**inputs):
    inp = np.asarray(inputs["input"], dtype=np.float32)
    dg = np.asarray(inputs["diagonal"], dtype=np.float32)
    assert inp.shape == (BATCH, IN_C, SIZE), inp.shape
    assert dg.shape == (OUT_C, IN_C, SIZE), dg.shape

    from concourse.bass_utils import run_bass_kernel_spmd

    nc = _get_nc()
    in_maps = prepare_in_maps(inp, dg)
    res = run_bass_kernel_spmd(nc, in_maps, list(range(N_CORES)))
    return assemble_output(res.results)


# revision 14
# speedup vs baseline: 1.2759x; 1.0527x over previous
"""Trainium2 Bass kernel for DiagonalMultiplySum.

out[b, o, s] = sum_i input[b, i, s] * diagonal[o, i, s]

Shapes (hardcoded): input (64, 256, 4096) f32, diagonal (256, 256, 4096) f32,
output (64, 256, 4096) f32.

Strategy: shard the size axis across 8 NeuronCores (512 positions per core).
Each position s is an independent matmul out[:, :, s] = diag[:, :, s] @ in[:, :, s]^T
with contraction over i (256 -> 2 chunks of 128 on the PE partition dim).

The kernel is HBM-DMA bound (~100.6 MB/core bf16 @ ~358 GB/s ~ 281 us), so
everything is organized around keeping the DMA stream dense:
  * bf16 I/O end-to-end (rel-err ~3e-3 << 2e-2 gate).
  * Host-side pre-packing: input+diagonal for each window are packed into a
    SINGLE contiguous 128-partition DRAM region -> one dma_start per window.
  * Output drained from PSUM as bf16 (DVE cast copy); host converts to fp32.
  * Loads ride the SP HWDGE ring, stores the ACT ring.
  * Window schedule: small windows at the start (quick pipeline ramp) and
    end (short tail), large windows in the middle (per-transfer efficiency,
    fewer read/write turnarounds).
"""

import os
import sys

for _p in ("/opt/trn_rl_repo",):
    if _p not in sys.path and os.path.isdir(_p):
        sys.path.insert(0, _p)

import numpy as np
from ml_dtypes import bfloat16

BATCH = 64
OUT_C = 256
IN_C = 256
SIZE = 4096
N_CORES = 8
S = SIZE // N_CORES  # 512 positions per core
P = 128


def _parse_sched(spec):
    out = []
    for seg in spec.split(","):
        seg = seg.strip()
        if "*" in seg:
            w, n = seg.split("*")
            out.extend([int(w)] * int(n))
        else:
            out.append(int(seg))
    assert sum(out) == S, (out, sum(out))
    assert all(w % 8 == 0 for w in out)
    return out

# window schedule: positions per window
WS = _parse_sched(os.environ.get("DMS_WS", "8,8,32*15,8,8"))
NW = len(WS)
# per-window free-dim extents (bf16 elems per partition)
X_IN = [2 * w * BATCH for w in WS]
X_DG = [w * 512 for w in WS]
X_LD = [a + b for a, b in zip(X_IN, X_DG)]
X_OUT = [2 * w * BATCH for w in WS]
LD_OFF = np.concatenate([[0], np.cumsum(X_LD)]).astype(int)
OUT_OFF = np.concatenate([[0], np.cumsum(X_OUT)]).astype(int)
TOT_LD = int(LD_OFF[-1])
TOT_OUT = int(OUT_OFF[-1])
S_OFF = np.concatenate([[0], np.cumsum(WS)]).astype(int)

# int8 output quantization: DEAD END, kept for reference.  The output
# distribution has fat tails (max |out| ~ 18.4, P(|out|>5) ~ 1e-3 — some
# input/diagonal rows are near-collinear), so any int8 clip blows the 2e-2
# error budget.
OUT8 = os.environ.get("DMS_OUT8", "0") == "1"
OUT_CLIP = float(os.environ.get("DMS_CLIP", "5.0"))
OUT_SCALE = 127.0 / OUT_CLIP

# Col-tiled compute (Layout B): input is the stationary operand ([i', b] =
# 128x64 per (position, ic)), diagonal is the moving operand ([i', (ob o')] =
# 128x256).  Even/odd positions map to PE column groups 0-63 / 64-127
# (tile_position (0,0) / (0,64)) so their streams run concurrently; psum
# holds 4 positions per bank: partitions (s%2)*64+b, free (pair, ob, o').
COLT = os.environ.get("DMS_COLT", "0") == "1"

_NC_CACHE = {}


def _build_nc():
    import concourse.bass as bass
    import concourse.mybir as mybir
    import concourse.tile as tile
    from contextlib import ExitStack

    fp32 = mybir.dt.float32
    bf16 = mybir.dt.bfloat16
    odt = mybir.dt.int8 if OUT8 else bf16
    nc = bass.Bass(trn_type="TRN2")

    # Host pre-packed layouts (see prepare_in_maps). Per window w the load
    # region holds  [ (ic, s, b) input | (s, ob, ic, o) diagonal ]  on each
    # of the 128 partitions (p = i % 128 for both halves).
    ld = nc.dram_tensor("loads", [P, TOT_LD], bf16, kind="ExternalInput")
    out = nc.dram_tensor("output", [P, TOT_OUT], odt, kind="ExternalOutput")

    n_ld_buf = int(os.environ.get("DMS_LDBUF", "4"))
    n_out_buf = int(os.environ.get("DMS_OUTBUF", "3"))
    n_ps_buf = int(os.environ.get("DMS_PSBUF", "8"))
    wmax = max(WS)

    with tile.TileContext(nc) as tc, ExitStack() as ctx:
        ld_pool = ctx.enter_context(tc.tile_pool(name="ldp", bufs=n_ld_buf))
        out_pool = ctx.enter_context(tc.tile_pool(name="outp", bufs=n_out_buf))
        ps_pool = ctx.enter_context(tc.tile_pool(name="psp", bufs=n_ps_buf, space="PSUM"))

        for w in range(NW):
            W = WS[w]
            ld_t = ld_pool.tile([P, max(X_LD)], bf16, name="ld_t", tag="ld_t")
            nc.sync.dma_start(
                out=ld_t[:, 0 : X_LD[w]], in_=ld[:, LD_OFF[w] : LD_OFF[w + 1]]
            )

            in_t4 = ld_t[:, 0 : X_IN[w]].rearrange(
                "p (ic s b) -> p ic s b", ic=2, b=BATCH
            )
            out_t = out_pool.tile([P, max(X_OUT)], odt, name="out_t", tag="out_t")

            if COLT:
                dg_t4 = ld_t[:, X_IN[w] : X_LD[w]].rearrange(
                    "p (s ic n) -> p s ic n", ic=2, n=2 * P
                )
                out_t3 = out_t[:, 0 : X_OUT[w]].rearrange("p (j n) -> p j n", n=512)
                for j4 in range(W // 4):
                    ps = ps_pool.tile([P, 512], fp32, name="ps")
                    for jj in range(2):
                        for sl in range(2):
                            s_loc = j4 * 4 + jj * 2 + sl
                            for ic in range(2):
                                nc.tensor.matmul(
                                    ps[sl * 64 : sl * 64 + 64, jj * 256 : jj * 256 + 256],
                                    in_t4[:, ic, s_loc, :],
                                    dg_t4[:, s_loc, ic, :],
                                    start=(ic == 0),
                                    stop=(ic == 1),
                                )
                    nc.vector.tensor_copy(out_t3[:, j4, :], ps)
            else:
                dg_t5 = ld_t[:, X_IN[w] : X_LD[w]].rearrange(
                    "p (s ob ic o) -> p s ob ic o", ob=2, ic=2, o=P
                )
                out_t4 = out_t[:, 0 : X_OUT[w]].rearrange(
                    "p (ob s b) -> p ob s b", ob=2, b=BATCH
                )
                for ob in range(2):
                    for s8 in range(W // 8):
                        ps = ps_pool.tile([P, 512], fp32, name="ps")
                        ps3 = ps.rearrange("p (q b) -> p q b", q=8)
                        for s_ib in range(8):
                            s_loc = s8 * 8 + s_ib
                            for ic in range(2):
                                nc.tensor.matmul(
                                    ps3[:, s_ib, :],
                                    dg_t5[:, s_loc, ob, ic, :],
                                    in_t4[:, ic, s_loc, :],
                                    start=(ic == 0),
                                    stop=(ic == 1),
                                )
                        # drain bank: psum (s, b) fp32 -> out_t (ob, s, b)
                        if OUT8:
                            nc.vector.tensor_scalar(
                                out_t4[:, ob, s8 * 8 : s8 * 8 + 8, :],
                                ps3,
                                OUT_SCALE,
                                None,
                                mybir.AluOpType.mult,
                            )
                        else:
                            nc.vector.tensor_copy(
                                out_t4[:, ob, s8 * 8 : s8 * 8 + 8, :], ps3
                            )

            nc.scalar.dma_start(
                out=out[:, OUT_OFF[w] : OUT_OFF[w + 1]], in_=out_t[:, 0 : X_OUT[w]]
            )

    _split_multi_waits(nc)
    return nc


def _split_multi_waits(nc):
    """Walrus codegen supports only ONE sync-wait per instruction.

    Tile emits multiple waits on some instructions; hoist all but the last
    onto same-engine NoOp instructions inserted immediately before the
    offender.  Per-engine in-order issue makes this exactly equivalent.
    """
    import concourse.mybir as mybir

    for f in nc.m.functions:
        for blk in f.blocks:
            new_list = []
            changed = False
            for inst in blk.instructions:
                si = inst.sync_info
                waits = list(si.on_wait) if si and si.on_wait else []
                if len(waits) > 1:
                    for w in waits[:-1]:
                        nop = mybir.InstNoOp(
                            name=nc.get_next_instruction_name(),
                            engine=inst.engine,
                            ins=[],
                            outs=[],
                            sync_info=mybir.SyncInfo(on_wait=[w], on_update=[]),
                        )
                        nc.register_instruction(nop)
                        new_list.append(nop)
                    si.on_wait = [waits[-1]]
                    changed = True
                new_list.append(inst)
            if changed:
                blk.instructions = new_list


def _get_nc():
    key = "nc"
    if key not in _NC_CACHE:
        _NC_CACHE[key] = _build_nc()
    return _NC_CACHE[key]


def prepare_in_maps(inp, dg):
    """Pack full fp32 inputs into per-core bf16 DRAM load regions."""
    in_maps = []
    for c in range(N_CORES):
        sl = slice(c * S, (c + 1) * S)
        # input [b, i, s] -> [p, s, ...],  i = ic*128 + p
        in_c = (
            inp[:, :, sl]
            .reshape(BATCH, 2, P, S)
            .transpose(2, 1, 3, 0)
            .astype(bfloat16)
        )  # [p, ic, s, b]
        # diagonal [o0, i, s],  o0 = ob*128 + o, i = ic*128 + p
        #   COLT:  -> [p, s, ic, ob, o]   (moving operand n = (ob, o))
        #   else:  -> [p, s, ob, ic, o]   (stationary [p, o] per (ob, ic))
        dg_c = (
            dg[:, :, sl]
            .reshape(2, P, 2, P, S)
            .transpose((3, 4, 2, 0, 1) if COLT else (3, 4, 0, 2, 1))
            .astype(bfloat16)
        )
        comb = np.empty((P, TOT_LD), dtype=bfloat16)
        for w in range(NW):
            s0, s1 = S_OFF[w], S_OFF[w + 1]
            o0, o1 = LD_OFF[w], LD_OFF[w + 1]
            comb[:, o0 : o0 + X_IN[w]] = in_c[:, :, s0:s1, :].reshape(P, -1)
            comb[:, o0 + X_IN[w] : o1] = dg_c[:, s0:s1].reshape(P, -1)
        in_maps.append({"loads": comb})
    return in_maps


def assemble_output(results):
    """Unpack per-core bf16 [p, (w: ob, s, b)] outputs to full fp32 [b, o, s]."""
    out = np.empty((BATCH, OUT_C, SIZE), dtype=np.float32)
    for c in range(N_CORES):
        sl = slice(c * S, (c + 1) * S)
        o_c = np.asarray(results[c]["output"])  # [P, TOT_OUT]
        if OUT8:
            o_c = o_c.astype(np.float32) * (1.0 / OUT_SCALE)
        if COLT:
            oc = np.empty((BATCH, OUT_C, S), dtype=np.float32)
            for w in range(NW):
                s0 = S_OFF[w]
                # [(sl b), (j4, jj, ob, o)] -> out[b, (ob o), s0 + 4*j4 + 2*jj + sl]
                blk = o_c[:, OUT_OFF[w] : OUT_OFF[w + 1]].reshape(
                    2, BATCH, WS[w] // 4, 2, 2, P
                )
                blk = blk.transpose(1, 4, 5, 2, 3, 0).reshape(BATCH, OUT_C, WS[w])
                oc[:, :, s0 : s0 + WS[w]] = blk.astype(np.float32)
            out[:, :, sl] = oc
        else:
            o_full = np.empty((P, 2, S, BATCH), dtype=np.float32)  # [p, ob, s, b]
            for w in range(NW):
                s0, s1 = S_OFF[w], S_OFF[w + 1]
                blk = o_c[:, OUT_OFF[w] : OUT_OFF[w + 1]].reshape(P, 2, WS[w], BATCH)
                o_full[:, :, s0:s1, :] = blk.astype(np.float32)
            # [p, ob, s, b] -> [b, (ob p), s]
            out[:, :, sl] = o_full.transpose(3, 1, 0, 2).reshape(BATCH, OUT_C, S)
    return out


def kernel(**inputs):
    inp = np.asarray(inputs["input"], dtype=np.float32)
    dg = np.asarray(inputs["diagonal"], dtype=np.float32)
    assert inp.shape == (BATCH, IN_C, SIZE), inp.shape
    assert dg.shape == (OUT_C, IN_C, SIZE), dg.shape

    from concourse.bass_utils import run_bass_kernel_spmd

    nc = _get_nc()
    in_maps = prepare_in_maps(inp, dg)
    res = run_bass_kernel_spmd(nc, in_maps, list(range(N_CORES)))
    return assemble_output(res.results)


# revision 16
# speedup vs baseline: 1.2796x; 1.0029x over previous
"""Trainium2 Bass kernel for DiagonalMultiplySum.

out[b, o, s] = sum_i input[b, i, s] * diagonal[o, i, s]

Shapes (hardcoded): input (64, 256, 4096) f32, diagonal (256, 256, 4096) f32,
output (64, 256, 4096) f32.

Strategy: shard the size axis across 8 NeuronCores (512 positions per core).
Each position s is an independent matmul out[:, :, s] = diag[:, :, s] @ in[:, :, s]^T
with contraction over i (256 -> 2 chunks of 128 on the PE partition dim).

The kernel is HBM-DMA bound (~100.6 MB/core bf16 @ ~358 GB/s ~ 281 us), so
everything is organized around keeping the DMA stream dense:
  * bf16 I/O end-to-end (rel-err ~3e-3 << 2e-2 gate).
  * Host-side pre-packing: input+diagonal for each window are packed into a
    SINGLE contiguous 128-partition DRAM region -> one dma_start per window.
  * Output drained from PSUM as bf16 (DVE cast copy); host converts to fp32.
  * Loads ride the SP HWDGE ring, stores the ACT ring.
  * Window schedule: small windows at the start (quick pipeline ramp) and
    end (short tail), large windows in the middle (per-transfer efficiency,
    fewer read/write turnarounds).
"""

import os
import sys

for _p in ("/opt/trn_rl_repo",):
    if _p not in sys.path and os.path.isdir(_p):
        sys.path.insert(0, _p)

import numpy as np
from ml_dtypes import bfloat16

BATCH = 64
OUT_C = 256
IN_C = 256
SIZE = 4096
N_CORES = 8
S = SIZE // N_CORES  # 512 positions per core
P = 128


def _parse_sched(spec):
    out = []
    for seg in spec.split(","):
        seg = seg.strip()
        if "*" in seg:
            w, n = seg.split("*")
            out.extend([int(w)] * int(n))
        else:
            out.append(int(seg))
    assert sum(out) == S, (out, sum(out))
    assert all(w % 8 == 0 for w in out)
    return out

# window schedule: positions per window
WS = _parse_sched(os.environ.get("DMS_WS", "8,8,16*30,8,8"))
NW = len(WS)
# per-window free-dim extents (bf16 elems per partition)
X_IN = [2 * w * BATCH for w in WS]
X_DG = [w * 512 for w in WS]
X_LD = [a + b for a, b in zip(X_IN, X_DG)]
X_OUT = [2 * w * BATCH for w in WS]
LD_OFF = np.concatenate([[0], np.cumsum(X_LD)]).astype(int)
OUT_OFF = np.concatenate([[0], np.cumsum(X_OUT)]).astype(int)
TOT_LD = int(LD_OFF[-1])
TOT_OUT = int(OUT_OFF[-1])
S_OFF = np.concatenate([[0], np.cumsum(WS)]).astype(int)

# int8 output quantization: DEAD END, kept for reference.  The output
# distribution has fat tails (max |out| ~ 18.4, P(|out|>5) ~ 1e-3 — some
# input/diagonal rows are near-collinear), so any int8 clip blows the 2e-2
# error budget.
OUT8 = os.environ.get("DMS_OUT8", "0") == "1"
OUT_CLIP = float(os.environ.get("DMS_CLIP", "5.0"))
OUT_SCALE = 127.0 / OUT_CLIP

# Col-tiled compute (Layout B): input is the stationary operand ([i', b] =
# 128x64 per (position, ic)), diagonal is the moving operand ([i', (ob o')] =
# 128x256).  Even/odd positions map to PE column groups 0-63 / 64-127
# (tile_position (0,0) / (0,64)) so their streams run concurrently; psum
# holds 4 positions per bank: partitions (s%2)*64+b, free (pair, ob, o').
COLT = os.environ.get("DMS_COLT", "1") == "1"

_NC_CACHE = {}


def _build_nc():
    import concourse.bass as bass
    import concourse.mybir as mybir
    import concourse.tile as tile
    from contextlib import ExitStack

    fp32 = mybir.dt.float32
    bf16 = mybir.dt.bfloat16
    odt = mybir.dt.int8 if OUT8 else bf16
    nc = bass.Bass(trn_type="TRN2")

    # Host pre-packed layouts (see prepare_in_maps). Per window w the load
    # region holds  [ (ic, s, b) input | (s, ob, ic, o) diagonal ]  on each
    # of the 128 partitions (p = i % 128 for both halves).
    ld = nc.dram_tensor("loads", [P, TOT_LD], bf16, kind="ExternalInput")
    out = nc.dram_tensor("output", [P, TOT_OUT], odt, kind="ExternalOutput")

    n_ld_buf = int(os.environ.get("DMS_LDBUF", "8"))
    n_out_buf = int(os.environ.get("DMS_OUTBUF", "6"))
    n_ps_buf = int(os.environ.get("DMS_PSBUF", "8"))
    wmax = max(WS)

    with tile.TileContext(nc) as tc, ExitStack() as ctx:
        ld_pool = ctx.enter_context(tc.tile_pool(name="ldp", bufs=n_ld_buf))
        out_pool = ctx.enter_context(tc.tile_pool(name="outp", bufs=n_out_buf))
        ps_pool = ctx.enter_context(tc.tile_pool(name="psp", bufs=n_ps_buf, space="PSUM"))

        for w in range(NW):
            W = WS[w]
            ld_t = ld_pool.tile([P, max(X_LD)], bf16, name="ld_t", tag="ld_t")
            nc.sync.dma_start(
                out=ld_t[:, 0 : X_LD[w]], in_=ld[:, LD_OFF[w] : LD_OFF[w + 1]]
            )

            in_t4 = ld_t[:, 0 : X_IN[w]].rearrange(
                "p (ic s b) -> p ic s b", ic=2, b=BATCH
            )
            out_t = out_pool.tile([P, max(X_OUT)], odt, name="out_t", tag="out_t")

            if COLT:
                dg_t4 = ld_t[:, X_IN[w] : X_LD[w]].rearrange(
                    "p (s ic n) -> p s ic n", ic=2, n=2 * P
                )
                out_t3 = out_t[:, 0 : X_OUT[w]].rearrange("p (j n) -> p j n", n=512)
                for j4 in range(W // 4):
                    ps = ps_pool.tile([P, 512], fp32, name="ps")
                    for jj in range(2):
                        for sl in range(2):
                            s_loc = j4 * 4 + jj * 2 + sl
                            for ic in range(2):
                                nc.tensor.matmul(
                                    ps[sl * 64 : sl * 64 + 64, jj * 256 : jj * 256 + 256],
                                    in_t4[:, ic, s_loc, :],
                                    dg_t4[:, s_loc, ic, :],
                                    start=(ic == 0),
                                    stop=(ic == 1),
                                )
                    nc.vector.tensor_copy(out_t3[:, j4, :], ps)
            else:
                dg_t5 = ld_t[:, X_IN[w] : X_LD[w]].rearrange(
                    "p (s ob ic o) -> p s ob ic o", ob=2, ic=2, o=P
                )
                out_t4 = out_t[:, 0 : X_OUT[w]].rearrange(
                    "p (ob s b) -> p ob s b", ob=2, b=BATCH
                )
                for ob in range(2):
                    for s8 in range(W // 8):
                        ps = ps_pool.tile([P, 512], fp32, name="ps")
                        ps3 = ps.rearrange("p (q b) -> p q b", q=8)
                        for s_ib in range(8):
                            s_loc = s8 * 8 + s_ib
                            for ic in range(2):
                                nc.tensor.matmul(
                                    ps3[:, s_ib, :],
                                    dg_t5[:, s_loc, ob, ic, :],
                                    in_t4[:, ic, s_loc, :],
                                    start=(ic == 0),
                                    stop=(ic == 1),
                                )
                        # drain bank: psum (s, b) fp32 -> out_t (ob, s, b)
                        if OUT8:
                            nc.vector.tensor_scalar(
                                out_t4[:, ob, s8 * 8 : s8 * 8 + 8, :],
                                ps3,
                                OUT_SCALE,
                                None,
                                mybir.AluOpType.mult,
                            )
                        else:
                            nc.vector.tensor_copy(
                                out_t4[:, ob, s8 * 8 : s8 * 8 + 8, :], ps3
                            )

            nc.scalar.dma_start(
                out=out[:, OUT_OFF[w] : OUT_OFF[w + 1]], in_=out_t[:, 0 : X_OUT[w]]
            )

    _split_multi_waits(nc)
    return nc


def _split_multi_waits(nc):
    """Walrus codegen supports only ONE sync-wait per instruction.

    Tile emits multiple waits on some instructions; hoist all but the last
    onto same-engine NoOp instructions inserted immediately before the
    offender.  Per-engine in-order issue makes this exactly equivalent.
    """
    import concourse.mybir as mybir

    for f in nc.m.functions:
        for blk in f.blocks:
            new_list = []
            changed = False
            for inst in blk.instructions:
                si = inst.sync_info
                waits = list(si.on_wait) if si and si.on_wait else []
                if len(waits) > 1:
                    for w in waits[:-1]:
                        nop = mybir.InstNoOp(
                            name=nc.get_next_instruction_name(),
                            engine=inst.engine,
                            ins=[],
                            outs=[],
                            sync_info=mybir.SyncInfo(on_wait=[w], on_update=[]),
                        )
                        nc.register_instruction(nop)
                        new_list.append(nop)
                    si.on_wait = [waits[-1]]
                    changed = True
                new_list.append(inst)
            if changed:
                blk.instructions = new_list


def _get_nc():
    key = "nc"
    if key not in _NC_CACHE:
        _NC_CACHE[key] = _build_nc()
    return _NC_CACHE[key]


def prepare_in_maps(inp, dg):
    """Pack full fp32 inputs into per-core bf16 DRAM load regions."""
    in_maps = []
    for c in range(N_CORES):
        sl = slice(c * S, (c + 1) * S)
        # input [b, i, s] -> [p, s, ...],  i = ic*128 + p
        in_c = (
            inp[:, :, sl]
            .reshape(BATCH, 2, P, S)
            .transpose(2, 1, 3, 0)
            .astype(bfloat16)
        )  # [p, ic, s, b]
        # diagonal [o0, i, s],  o0 = ob*128 + o, i = ic*128 + p
        #   COLT:  -> [p, s, ic, ob, o]   (moving operand n = (ob, o))
        #   else:  -> [p, s, ob, ic, o]   (stationary [p, o] per (ob, ic))
        dg_c = (
            dg[:, :, sl]
            .reshape(2, P, 2, P, S)
            .transpose((3, 4, 2, 0, 1) if COLT else (3, 4, 0, 2, 1))
            .astype(bfloat16)
        )
        comb = np.empty((P, TOT_LD), dtype=bfloat16)
        for w in range(NW):
            s0, s1 = S_OFF[w], S_OFF[w + 1]
            o0, o1 = LD_OFF[w], LD_OFF[w + 1]
            comb[:, o0 : o0 + X_IN[w]] = in_c[:, :, s0:s1, :].reshape(P, -1)
            comb[:, o0 + X_IN[w] : o1] = dg_c[:, s0:s1].reshape(P, -1)
        in_maps.append({"loads": comb})
    return in_maps


def assemble_output(results):
    """Unpack per-core bf16 [p, (w: ob, s, b)] outputs to full fp32 [b, o, s]."""
    out = np.empty((BATCH, OUT_C, SIZE), dtype=np.float32)
    for c in range(N_CORES):
        sl = slice(c * S, (c + 1) * S)
        o_c = np.asarray(results[c]["output"])  # [P, TOT_OUT]
        if OUT8:
            o_c = o_c.astype(np.float32) * (1.0 / OUT_SCALE)
        if COLT:
            oc = np.empty((BATCH, OUT_C, S), dtype=np.float32)
            for w in range(NW):
                s0 = S_OFF[w]
                # [(sl b), (j4, jj, ob, o)] -> out[b, (ob o), s0 + 4*j4 + 2*jj + sl]
                blk = o_c[:, OUT_OFF[w] : OUT_OFF[w + 1]].reshape(
                    2, BATCH, WS[w] // 4, 2, 2, P
                )
                blk = blk.transpose(1, 4, 5, 2, 3, 0).reshape(BATCH, OUT_C, WS[w])
                oc[:, :, s0 : s0 + WS[w]] = blk.astype(np.float32)
            out[:, :, sl] = oc
        else:
            o_full = np.empty((P, 2, S, BATCH), dtype=np.float32)  # [p, ob, s, b]
            for w in range(NW):
                s0, s1 = S_OFF[w], S_OFF[w + 1]
                blk = o_c[:, OUT_OFF[w] : OUT_OFF[w + 1]].reshape(P, 2, WS[w], BATCH)
                o_full[:, :, s0:s1, :] = blk.astype(np.float32)
            # [p, ob, s, b] -> [b, (ob p), s]
            out[:, :, sl] = o_full.transpose(3, 1, 0, 2).reshape(BATCH, OUT_C, S)
    return out


def kernel(**inputs):
    inp = np.asarray(inputs["input"], dtype=np.float32)
    dg = np.asarray(inputs["diagonal"], dtype=np.float32)
    assert inp.shape == (BATCH, IN_C, SIZE), inp.shape
    assert dg.shape == (OUT_C, IN_C, SIZE), dg.shape

    from concourse.bass_utils import run_bass_kernel_spmd

    nc = _get_nc()
    in_maps = prepare_in_maps(inp, dg)
    res = run_bass_kernel_spmd(nc, in_maps, list(range(N_CORES)))
    return assemble_output(res.results)
